# revision 41
# baseline (speedup 1.0000x reference)
"""Causal multi-head attention (PBrelax) for TRN2, sharded over 8 NeuronCores.

Sharding: batch (2) x head-group (4 heads each) = 8 shards, one per core.
Each core computes q/k/v projections for its 256 channels, causal
flash-style attention in S^T layout (keys on partitions), and a partial
output projection; the host sums the 4 per-batch partials and adds bp.

The global abs-max subtraction in PBrelax is softmax-shift-invariant, so it
is mathematically a no-op on the output; logits are bounded (~|x|<4) so
exp() without max-subtraction is numerically safe.

Schedule (v2): the attention phase is ACT(exp)-bound per strip while the
projections are PE-bound, so the kernel runs a skewed software pipeline:
stream(h) emits head h's S^T+exp+mask interleaved with ready PE work —
head h-1's AV pieces + per-region normalize (region-major, so PSUM holds
one [65,512] region at a time instead of a [65,T] head) and the v
projection (head 0's stream). The output projection is split by head
pair into two bf16 partials summed on host, so its first half runs amid
streams h2/h3. Across reps, the last head's AV/norm/output drain is
carried into the next rep's q/k-projection preamble (cross-rep software
pipelining); x loads are dispatched from the Pool queue so they land
during the previous rep's tail. Causal trimming skips all
below-diagonal strip columns (split stop flags finalize each PSUM
column block at its true last accumulation). Engines: exp on ACT, q/k
bias eviction on ACT, everything PSUM-touching else on DVE (GPSIMD
cannot read PSUM), SBUF-only mask multiplies on Pool.
PSUM budget: ps strips 2x2 banks, py regions 2x1, aux (proj/pv/rb/pot)
2x1 = 8 banks.

fp8 was evaluated and rejected: for random-sign dot products the output
relative error equals the per-element quantization error (~5% for
e4m3, no sqrt-N averaging), which would blow the 2e-2 gate; bf16
(~0.3%) is the floor dtype here.
"""

import numpy as np
import ml_dtypes

import concourse.bass as bass
import concourse.bacc as bacc
import concourse.mybir as mybir
import concourse.tile as tile

BF16 = mybir.dt.bfloat16
F32 = mybir.dt.float32
F32R = mybir.dt.float32r
EXP = mybir.ActivationFunctionType.Exp

B, T_FULL, C, H = 2, 2048, 1024, 16
HD = 64
NH = 4            # heads per core
CS = NH * HD      # 256 channels per core
P = 128
IC = 512          # attention region width (= one PSUM bank of f32)
KF = C // P       # 8 contraction chunks
LSCALE = 0.125    # (1/(alpha*sqrt(hd))) * alpha = 1/8
N_CORES = 8


def build_nc(T=T_FULL, nstrip=1024, reps=1, mask_eng="pool", rbs_eng="dve"):
    NJ = T // P
    ICе = min(IC, T)
    R = ICе // P
    NR = T // ICе     # regions per head
    nc = bacc.Bacc(target_bir_lowering=False)

    xq = nc.dram_tensor("xq", [C, T], BF16, kind="ExternalInput")
    xk = nc.dram_tensor("xk", [C, T], BF16, kind="ExternalInput")
    xv = nc.dram_tensor("xv", [C, T], BF16, kind="ExternalInput")
    wq = nc.dram_tensor("wq", [C, CS], BF16, kind="ExternalInput")
    wk = nc.dram_tensor("wk", [C, CS], BF16, kind="ExternalInput")
    wv = nc.dram_tensor("wv", [C, NH * 65], BF16, kind="ExternalInput")
    wp = nc.dram_tensor("wp", [CS, C], BF16, kind="ExternalInput")
    bq2 = nc.dram_tensor("bq2", [P, 2], F32, kind="ExternalInput")
    bk2 = nc.dram_tensor("bk2", [P, 2], F32, kind="ExternalInput")
    bv260 = nc.dram_tensor("bv260", [P, NH * 65], F32, kind="ExternalInput")
    msk = nc.dram_tensor("msk", [P, R * ICе], BF16, kind="ExternalInput")
    ones64 = nc.dram_tensor("ones64", [1, HD], F32R, kind="ExternalInput")
    # output split by head-pair: out = out0 + out1 summed on host (lets the
    # heads-0/1 half of the output projection run before heads 2/3 finish)
    out0 = nc.dram_tensor("out0", [T, C], BF16, kind="ExternalOutput")
    out1 = nc.dram_tensor("out1", [T, C], BF16, kind="ExternalOutput")

    with tile.TileContext(nc) as tc:
        with tc.tile_pool(name="sb", bufs=1) as sb, \
             tc.tile_pool(name="xp", bufs=2) as xp, \
             tc.tile_pool(name="es", bufs=2 * NJ) as ea, \
             tc.tile_pool(name="nrm", bufs=3) as nrm:

            def load_x(xd, after_first=None):
                """Load x chunks; optionally emit (weight, bias) DMAs right
                after chunk 0 so the first matmul isn't gated on the full x."""
                xm = xp.tile([P, KF * T], BF16, tag="x", name="xm")
                for kc in range(KF):
                    nc.gpsimd.dma_start(xm[:, kc * T:(kc + 1) * T],
                                        xd[kc * P:(kc + 1) * P, :])
                    if kc == 0 and after_first is not None:
                        after_first()
                return xm

            # ---- weights / constants; xk0+wk+bk first so k-proj starts ASAP
            wk_m = sb.tile([P, KF * CS], BF16)
            bk_d = sb.tile([P, 2], F32)

            def _wk_dmas():
                nc.sync.dma_start(wk_m.rearrange("p (c n) -> p c n", c=KF),
                                  wk[:, :].rearrange("(c p) n -> p c n", p=P))
                nc.sync.dma_start(bk_d, bk2[:, :])
            xkm = load_x(xk, _wk_dmas)
            wq_m = sb.tile([P, KF * CS], BF16)
            bq_d = sb.tile([P, 2], F32)

            def _wq_dmas():
                nc.sync.dma_start(wq_m.rearrange("p (c n) -> p c n", c=KF),
                                  wq[:, :].rearrange("(c p) n -> p c n", p=P))
                nc.sync.dma_start(bq_d, bq2[:, :])
            xqm = load_x(xq, _wq_dmas)
            wv_m = sb.tile([P, KF * NH * 65], BF16)
            bv_d = sb.tile([P, NH * 65], F32)

            def _wv_dmas():
                nc.sync.dma_start(wv_m.rearrange("p (c n) -> p c n", c=KF),
                                  wv[:, :].rearrange("(c p) n -> p c n", p=P))
                nc.sync.dma_start(bv_d, bv260[:, :])
            xvm = load_x(xv, _wv_dmas)
            msk_d = sb.tile([P, R * ICе], BF16)
            nc.sync.dma_start(msk_d, msk[:, :])
            wp_s = sb.tile([P, 2 * C], BF16)
            nc.sync.dma_start(wp_s.rearrange("p (c n) -> p c n", c=2),
                              wp[:, :].rearrange("(c p) n -> p c n", p=P))
            one_s = sb.tile([1, HD], F32R)
            nc.sync.dma_start(one_s, ones64[:, :])
            # prime the ACT exp table during the DMA preamble so the first
            # attention strip doesn't pay the 1.3us table load
            warm = sb.tile([1, 2], F32)
            nc.vector.memset(warm, 0.0)
            warm2 = sb.tile([1, 2], BF16)
            nc.scalar.activation(warm2, warm, EXP, scale=1.0)
            # pre-touch constants on DVE so downstream DVE consumers need no
            # extra cross-engine waits (walrus sync-wait slots are scarce)
            bq_s = sb.tile([P, 2], F32)
            nc.vector.tensor_copy(bq_s, bq_d)
            bk_s = sb.tile([P, 2], F32)
            nc.vector.tensor_copy(bk_s, bk_d)
            bv_s = sb.tile([P, NH * 65], F32)
            nc.vector.tensor_copy(bv_s, bv_d)
            msk_s = sb.tile([P, R * ICе], BF16)
            nc.vector.tensor_copy(msk_s, msk_d)

            carry = []       # previous rep's drain units (run in our preamble)
            pend = [None]    # (h, r, py_r, rh) awaiting norm_post
            with tc.tile_pool(name="pap", bufs=1, space="PSUM") as pa, \
                 tc.tile_pool(name="osb", bufs=2) as ob:
              for rep in range(reps):
                if rep > 0:
                    xkm = load_x(xk)
                    xqm = load_x(xq)
                    xvm = load_x(xv)
                qT_s = sb.tile([P, 2 * T], BF16)
                kT_s = sb.tile([P, 2 * T], BF16)
                v_s = sb.tile([P, NJ * 260], BF16)
                yT_s = sb.tile([P, 2 * T], BF16)

                if True:

                    def proj_qk_tile(w_m, b_t, x_m, out_s, dt, t0):
                        """One [P, 512] q/k projection tile: 8-chunk matmul
                        accumulate + DVE bias eviction into out_s dt half."""
                        ps = pa.tile([P, 512], F32, tag="aux", bufs=2,
                                     name="pj")
                        for kc in range(KF):
                            lhsT = w_m[:, kc * CS + dt * P: kc * CS + dt * P + P]
                            nc.tensor.matmul(
                                ps, lhsT,
                                x_m[:, kc * T + t0: kc * T + t0 + 512],
                                start=(kc == 0), stop=(kc == KF - 1))
                        nc.scalar.add(
                            out_s[:, dt * T + t0: dt * T + t0 + 512],
                            ps, b_t[:, dt:dt + 1])

                    def proj_v_tile(jt):
                        """One [P, 260] v-projection chunk (tokens jt*P..)."""
                        pv = pa.tile([P, NH * 65], F32, tag="aux", bufs=2,
                                     name="pv")
                        for kc in range(KF):
                            nc.tensor.matmul(
                                pv, xvm[:, kc * T + jt * P: kc * T + (jt + 1) * P],
                                wv_m[:, kc * NH * 65:(kc + 1) * NH * 65],
                                start=(kc == 0), stop=(kc == KF - 1))
                        nc.vector.tensor_add(v_s[:, jt * 260:(jt + 1) * 260],
                                             pv, bv_s)

                    # ---- pre-attention: full q/k projections (k first; its
                    # dt1 half fills PE while the xq DMA stream lands), with
                    # the previous rep's drain units interleaved between tiles
                    projs = [
                        (lambda dt=dt, th=th: proj_qk_tile(
                            wk_m, bk_s, xkm, kT_s, dt, th * 512))
                        for dt in range(2) for th in range(T // 512)
                    ] + [
                        (lambda dt=dt, th=th: proj_qk_tile(
                            wq_m, bq_s, xqm, qT_s, dt, th * 512))
                        for dt in range(2) for th in range(T // 512)
                    ]
                    cdone = 0
                    for i, pt in enumerate(projs):
                        want = (len(carry) * (i + 1)) // len(projs)
                        while cdone < want:
                            carry[cdone]()
                            cdone += 1
                        pt()
                    for u in carry[cdone:]:
                        u()
                    carry = []

                    # ---- PE filler queue: work metered into attention stalls
                    fillers = [lambda jt=jt: proj_v_tile(jt)
                               for jt in range(NJ)]
                    fillers.reverse()      # pop() from the front of the list

                    def emit_filler(n=1):
                        for _ in range(n):
                            if fillers:
                                fillers.pop()()

                    def st_phase(h, jc, between=None):
                        """S^T + exp + mask for (head h, key chunk jc).
                        Returns es_map entry: list of (c0, cw, es_tile)."""
                        ht, hr = h // 2, (h % 2) * 64
                        ic0 = jc // R
                        d0 = (jc % R) * P
                        entries = []
                        for s in range((T - ic0 * ICе + nstrip - 1) // nstrip):
                            if s > 0 and between is not None:
                                between()
                            c0 = ic0 * ICе + s * nstrip
                            cw = min(nstrip, T - c0)
                            ps = pa.tile([P, nstrip], F32, tag="ps", bufs=2,
                                         name="pst")
                            q0 = d0 if s == 0 else 0
                            while q0 < cw:
                                qw = min(512 - q0 % 512, cw - q0)
                                nc.tensor.matmul(
                                    ps[:, q0:q0 + qw],
                                    kT_s[hr:hr + 64, ht * T + jc * P: ht * T + (jc + 1) * P],
                                    qT_s[hr:hr + 64, ht * T + c0 + q0: ht * T + c0 + q0 + qw],
                                    start=True, stop=True)
                                q0 += qw
                            es = ea.tile([P, nstrip], BF16, tag="es", name="es")
                            e0 = d0 if s == 0 else 0
                            nc.scalar.activation(es[:, e0:cw], ps[:, e0:cw],
                                                 EXP, scale=LSCALE)
                            if s == 0 and d0 < ICе:
                                cp_tog[0] ^= 1
                                me = {"pool": nc.gpsimd, "dve": nc.vector}.get(
                                    mask_eng,
                                    nc.vector if cp_tog[0] else nc.gpsimd)
                                me.tensor_mul(
                                    es[:, d0:ICе], es[:, d0:ICе],
                                    msk_s[:, (jc % R) * ICе + d0:(jc % R + 1) * ICе])
                            entries.append((c0, cw, es))
                        return entries

                    def av_piece(h, r, jc, es_map, py_r):
                        """One key chunk's contribution to region r of head h."""
                        m = jc % R
                        vv = v_s[:, jc * 260 + h * 65: jc * 260 + h * 65 + 65]
                        # locate region r's columns in jc's es tiles
                        ecol = None
                        for (c0, cw, es) in es_map[jc]:
                            if c0 <= r * ICе < c0 + cw:
                                ecol = (es, r * ICе - c0)
                                break
                        es, off = ecol
                        if jc // R == r:
                            # diagonal chunk: cols [m*P, ICе); col block m
                            # ends its accumulation here (split stop flag)
                            nc.tensor.matmul(
                                py_r[:, m * P:(m + 1) * P],
                                vv, es[:, off + m * P: off + (m + 1) * P],
                                start=(jc == 0), stop=True)
                            if (m + 1) * P < ICе:
                                nc.tensor.matmul(
                                    py_r[:, (m + 1) * P:ICе],
                                    vv, es[:, off + (m + 1) * P: off + ICе],
                                    start=(jc == 0), stop=False)
                        else:
                            nc.tensor.matmul(
                                py_r[:, 0:ICе],
                                vv, es[:, off: off + ICе],
                                start=(jc == 0), stop=(jc == R * r + R - 1))

                    def norm_pre(py_r):
                        """Reciprocal of region denominator (DVE), issued
                        right after the region's AV so it runs under other
                        PE work."""
                        rh = nrm.tile([1, ICе], F32R, tag="rh", name="rh")
                        with nc.allow_low_precision(reason="f32r row-scale"):
                            nc.vector.reciprocal(rh, py_r[64:65, :])
                        return rh

                    def norm_post(h, r, py_r, rh):
                        """Broadcast 1/denom and scale region r into yT; then
                        kick the output projection half whose heads are done."""
                        ht, hr = h // 2, (h % 2) * 64
                        rb = pa.tile([HD, ICе], F32, tag="aux", bufs=2,
                                     name="rb")
                        nc.tensor.matmul(rb, one_s, rh, start=True, stop=True)
                        rbs = nrm.tile([HD, ICе], F32, tag="rbs", name="rbs")
                        if rbs_eng == "act":
                            nc.scalar.copy(rbs, rb)
                        else:
                            nc.vector.tensor_copy(rbs, rb)
                        b0 = r * ICе
                        nc.vector.tensor_mul(
                            yT_s[hr:hr + 64, ht * T + b0: ht * T + b0 + ICе],
                            py_r[0:64, :], rbs)
                        if h == 1:
                            out_proj_cc(0, r)
                        elif h == 3:
                            out_proj_cc(1, r)

                    cp_tog = [0]

                    def out_proj_cc(ct, cc):
                        """Half output projection (head pair ct) for token
                        chunk cc; evictions alternate DVE/Pool. ct0 runs amid
                        the attention streams (aux psum, 512-wide); ct1 runs
                        in the drain where the strip pool is idle (1024-wide,
                        one eviction per token tile)."""
                        outd = out0 if ct == 0 else out1
                        for it in range(cc * ICе // P, (cc + 1) * ICе // P):
                            ot = ob.tile([P, C], BF16, tag="ot", name="ot")
                            for nn in range(2):
                                pot = pa.tile([P, 512], F32, tag="aux",
                                              bufs=2, name="pot")
                                nc.tensor.matmul(
                                    pot,
                                    yT_s[:, ct * T + it * P: ct * T + (it + 1) * P],
                                    wp_s[:, ct * C + nn * 512: ct * C + (nn + 1) * 512],
                                    start=True, stop=True)
                                if ct == 1:
                                    cp_tog[0] ^= 1
                                    if cp_tog[0]:
                                        nc.scalar.copy(
                                            ot[:, nn * 512:(nn + 1) * 512], pot)
                                    else:
                                        nc.vector.tensor_copy(
                                            ot[:, nn * 512:(nn + 1) * 512], pot)
                                else:
                                    nc.vector.tensor_copy(
                                        ot[:, nn * 512:(nn + 1) * 512], pot)
                            nc.sync.dma_start(outd[it * P:(it + 1) * P, :], ot)

                    # ---- skewed pipeline over heads ----

                    def make_av_units(ph, pmap):
                        """Work-unit closures for head ph's AV + norms, in
                        accumulation order (region-major, jc ascending)."""
                        units = []
                        state = {}

                        def start_region(r):
                            state[r] = pa.tile([65, ICе], F32, tag="py",
                                               bufs=2, name="py")

                        for r in range(NR):
                            for jc in range(R * (r + 1)):
                                def u(r=r, jc=jc):
                                    if jc == 0:
                                        start_region(r)
                                    av_piece(ph, r, jc, pmap, state[r])
                                units.append(u)

                            def un(r=r):
                                rh = norm_pre(state[r])
                                if pend[0] is not None:
                                    norm_post(*pend[0])
                                pend[0] = (ph, r, state[r], rh)
                            units.append(un)
                        return units

                    prev_units = []
                    for h in range(NH):
                        es_map = {}
                        units = prev_units + fillers[::-1]
                        fillers.clear()
                        done = 0
                        prog = [0]

                        def pump():
                            prog[0] += 1
                            want = (len(units) * prog[0] + NSTEP - 1) // NSTEP
                            while done_[0] < min(want, len(units)):
                                units[done_[0]]()
                                done_[0] += 1

                        done_ = [0]
                        NSTEP = sum((T - (jc // R) * ICе + nstrip - 1) // nstrip
                                    for jc in range(NJ))
                        for jc in range(NJ):
                            es_map[jc] = st_phase(h, jc, between=pump)
                            pump()
                        done = done_[0]
                        for u in units[done:]:
                            u()
                        prev_units = make_av_units(h, es_map)

                    # last head's AV+norm becomes the next rep's preamble
                    # filler (or the final drain on the last rep)
                    def flush_pend():
                        if pend[0] is not None:
                            norm_post(*pend[0])
                            pend[0] = None
                    carry = prev_units + [flush_pend]

              for u in carry:
                  u()

    return nc


def make_core_inputs(query, key, value, Wq, bq, Wk, bk, Wv, bv, Wp, T=T_FULL):
    """Host-side shard prep. Returns list of 8 in_maps (bf16 numpy)."""
    bf = ml_dtypes.bfloat16
    query = np.asarray(query, np.float32)
    key = np.asarray(key, np.float32)
    value = np.asarray(value, np.float32)
    Wq, bq = np.asarray(Wq, np.float32), np.asarray(bq, np.float32)
    Wk, bk = np.asarray(Wk, np.float32), np.asarray(bk, np.float32)
    Wv, bv = np.asarray(Wv, np.float32), np.asarray(bv, np.float32)
    Wp = np.asarray(Wp, np.float32)

    ICе = min(IC, T)
    R = ICе // P
    jj = np.arange(P)[:, None]
    cc = np.arange(ICе)[None, :]
    msk_np = np.concatenate(
        [(cc >= (128 * m + jj)) for m in range(R)], axis=1).astype(bf)
    ones64 = np.ones((1, HD), np.float32)

    xT = {}
    for nm, x in (("q", query), ("k", key), ("v", value)):
        for b in range(B):
            xT[nm, b] = np.ascontiguousarray(x[b].T).astype(bf)

    in_maps = []
    for core in range(N_CORES):
        b, g = core // 4, core % 4
        hs = slice(g * CS, (g + 1) * CS)
        wv_p = np.zeros((C, NH * 65), np.float32)
        bv_p = np.zeros((P, NH * 65), np.float32)
        wv_h = Wv[:, hs]
        for h in range(NH):
            wv_p[:, h * 65:h * 65 + 64] = wv_h[:, h * 64:(h + 1) * 64]
            bv_p[:, h * 65:h * 65 + 64] = bv[hs][h * 64:(h + 1) * 64][None, :]
            bv_p[:, h * 65 + 64] = 1.0
        in_maps.append(dict(
            xq=xT["q", b], xk=xT["k", b], xv=xT["v", b],
            wq=Wq[:, hs].astype(bf), wk=Wk[:, hs].astype(bf),
            wv=wv_p.astype(bf), wp=Wp[hs, :].astype(bf),
            bq2=np.ascontiguousarray(bq[hs].reshape(2, P).T),
            bk2=np.ascontiguousarray(bk[hs].reshape(2, P).T),
            bv260=bv_p, msk=msk_np, ones64=ones64))
    return in_maps


_NC = None
TRACE = False          # set True (e.g. from test.py) to neuron-profile the run
LAST = None            # BassKernelResults of the most recent kernel() call


def kernel(query, key, value, att_mask, Wq, bq, Wk, bk, Wv, bv, Wp, bp):
    from concourse.bass_utils import run_bass_kernel_spmd
    global _NC, LAST
    if _NC is None:
        _NC = build_nc()
        _NC.finalize()
    in_maps = make_core_inputs(query, key, value, Wq, bq, Wk, bk, Wv, bv, Wp)
    try:
        res = run_bass_kernel_spmd(_NC, in_maps, core_ids=list(range(N_CORES)),
                                   trace=TRACE)
    except Exception:
        # transient axon-tunnel desyncs happen; one retry is usually enough
        res = run_bass_kernel_spmd(_NC, in_maps, core_ids=list(range(N_CORES)),
                                   trace=TRACE)
    LAST = res
    full = np.zeros((B, T_FULL, C), np.float32)
    for core in range(N_CORES):
        full[core // 4] += res.results[core]["out0"].astype(np.float32)
        full[core // 4] += res.results[core]["out1"].astype(np.float32)
    full += np.asarray(bp, np.float32)[None, None, :]
    return full


# revision 42
# speedup vs baseline: 1.1891x; 1.1891x over previous
"""Causal multi-head attention (PBrelax) for TRN2, sharded over 8 NeuronCores.

Sharding: batch (2) x head-group (4 heads each) = 8 shards, one per core.
Each core computes q/k/v projections for its 256 channels, causal
flash-style attention in S^T layout (keys on partitions), and a partial
output projection; the host sums the 4 per-batch partials and adds bp.

The global abs-max subtraction in PBrelax is softmax-shift-invariant, so it
is mathematically a no-op on the output; logits are bounded (~|x|<4) so
exp() without max-subtraction is numerically safe.

Schedule (v2): the attention phase is ACT(exp)-bound per strip while the
projections are PE-bound, so the kernel runs a skewed software pipeline:
stream(h) emits head h's S^T+exp+mask interleaved with ready PE work —
head h-1's AV pieces + per-region normalize (region-major, so PSUM holds
one [65,512] region at a time instead of a [65,T] head) and the v
projection (head 0's stream). The output projection is split by head
pair into two bf16 partials summed on host, so its first half runs amid
streams h2/h3. Across reps, the last head's AV/norm/output drain is
carried into the next rep's q/k-projection preamble (cross-rep software
pipelining); x loads are dispatched from the Pool queue so they land
during the previous rep's tail. Causal trimming skips all
below-diagonal strip columns (split stop flags finalize each PSUM
column block at its true last accumulation). Engines: exp on ACT, q/k
bias eviction on ACT, everything PSUM-touching else on DVE (GPSIMD
cannot read PSUM), SBUF-only mask multiplies on Pool.
PSUM budget: ps strips 2x2 banks, py regions 2x1, aux (proj/pv/rb/pot)
2x1 = 8 banks.

fp8 was evaluated and rejected: for random-sign dot products the output
relative error equals the per-element quantization error (~5% for
e4m3, no sqrt-N averaging), which would blow the 2e-2 gate; bf16
(~0.3%) is the floor dtype here.
"""

import numpy as np
import ml_dtypes

import concourse.bass as bass
import concourse.bacc as bacc
import concourse.mybir as mybir
import concourse.tile as tile

BF16 = mybir.dt.bfloat16
F32 = mybir.dt.float32
F32R = mybir.dt.float32r
EXP = mybir.ActivationFunctionType.Exp

B, T_FULL, C, H = 2, 2048, 1024, 16
HD = 64
NH = 4            # heads per core
CS = NH * HD      # 256 channels per core
P = 128
IC = 512          # attention region width (= one PSUM bank of f32)
KF = C // P       # 8 contraction chunks
LSCALE = 0.125    # (1/(alpha*sqrt(hd))) * alpha = 1/8
N_CORES = 8


def build_nc(T=T_FULL, nstrip=1024, reps=1, mask_eng="pool", rbs_eng="dve"):
    NJ = T // P
    ICе = min(IC, T)
    R = ICе // P
    NR = T // ICе     # regions per head
    nc = bacc.Bacc(target_bir_lowering=False)

    xq = nc.dram_tensor("xq", [C, T], BF16, kind="ExternalInput")
    xk = nc.dram_tensor("xk", [C, T], BF16, kind="ExternalInput")
    xv = nc.dram_tensor("xv", [C, T], BF16, kind="ExternalInput")
    wq = nc.dram_tensor("wq", [C, CS], BF16, kind="ExternalInput")
    wk = nc.dram_tensor("wk", [C, CS], BF16, kind="ExternalInput")
    wv = nc.dram_tensor("wv", [C, NH * 65], BF16, kind="ExternalInput")
    wp = nc.dram_tensor("wp", [CS, C], BF16, kind="ExternalInput")
    bq2 = nc.dram_tensor("bq2", [P, 2], F32, kind="ExternalInput")
    bk2 = nc.dram_tensor("bk2", [P, 2], F32, kind="ExternalInput")
    bv260 = nc.dram_tensor("bv260", [P, NH * 65], F32, kind="ExternalInput")
    msk = nc.dram_tensor("msk", [P, R * ICе], BF16, kind="ExternalInput")
    ones64 = nc.dram_tensor("ones64", [1, HD], F32R, kind="ExternalInput")
    out = nc.dram_tensor("out", [T, C], BF16, kind="ExternalOutput")

    with tile.TileContext(nc) as tc:
        with tc.tile_pool(name="sb", bufs=1) as sb, \
             tc.tile_pool(name="xp", bufs=2) as xp, \
             tc.tile_pool(name="es", bufs=2 * NJ) as ea, \
             tc.tile_pool(name="nrm", bufs=3) as nrm:

            def load_x(xd, after_first=None):
                """Load x chunks; optionally emit (weight, bias) DMAs right
                after chunk 0 so the first matmul isn't gated on the full x."""
                xm = xp.tile([P, KF * T], BF16, tag="x", name="xm")
                for kc in range(KF):
                    nc.gpsimd.dma_start(xm[:, kc * T:(kc + 1) * T],
                                        xd[kc * P:(kc + 1) * P, :])
                    if kc == 0 and after_first is not None:
                        after_first()
                return xm

            # ---- weights / constants; xk0+wk+bk first so k-proj starts ASAP
            wk_m = sb.tile([P, KF * CS], BF16)
            bk_d = sb.tile([P, 2], F32)

            def _wk_dmas():
                nc.sync.dma_start(wk_m.rearrange("p (c n) -> p c n", c=KF),
                                  wk[:, :].rearrange("(c p) n -> p c n", p=P))
                nc.sync.dma_start(bk_d, bk2[:, :])
            xkm = load_x(xk, _wk_dmas)
            wq_m = sb.tile([P, KF * CS], BF16)
            bq_d = sb.tile([P, 2], F32)

            def _wq_dmas():
                nc.sync.dma_start(wq_m.rearrange("p (c n) -> p c n", c=KF),
                                  wq[:, :].rearrange("(c p) n -> p c n", p=P))
                nc.sync.dma_start(bq_d, bq2[:, :])
            xqm = load_x(xq, _wq_dmas)
            wv_m = sb.tile([P, KF * NH * 65], BF16)
            bv_d = sb.tile([P, NH * 65], F32)

            def _wv_dmas():
                nc.sync.dma_start(wv_m.rearrange("p (c n) -> p c n", c=KF),
                                  wv[:, :].rearrange("(c p) n -> p c n", p=P))
                nc.sync.dma_start(bv_d, bv260[:, :])
            xvm = load_x(xv, _wv_dmas)
            msk_d = sb.tile([P, R * ICе], BF16)
            nc.sync.dma_start(msk_d, msk[:, :])
            wp_s = sb.tile([P, 2 * C], BF16)
            nc.sync.dma_start(wp_s.rearrange("p (c n) -> p c n", c=2),
                              wp[:, :].rearrange("(c p) n -> p c n", p=P))
            one_s = sb.tile([1, HD], F32R)
            nc.sync.dma_start(one_s, ones64[:, :])
            # prime the ACT exp table during the DMA preamble so the first
            # attention strip doesn't pay the 1.3us table load
            warm = sb.tile([1, 2], F32)
            nc.vector.memset(warm, 0.0)
            warm2 = sb.tile([1, 2], BF16)
            nc.scalar.activation(warm2, warm, EXP, scale=1.0)
            # pre-touch constants on DVE so downstream DVE consumers need no
            # extra cross-engine waits (walrus sync-wait slots are scarce)
            bq_s = sb.tile([P, 2], F32)
            nc.vector.tensor_copy(bq_s, bq_d)
            bk_s = sb.tile([P, 2], F32)
            nc.vector.tensor_copy(bk_s, bk_d)
            bv_s = sb.tile([P, NH * 65], F32)
            nc.vector.tensor_copy(bv_s, bv_d)
            msk_s = sb.tile([P, R * ICе], BF16)
            nc.vector.tensor_copy(msk_s, msk_d)

            carry = []       # previous rep's drain units (run in our preamble)
            pend = [None]    # (h, r, py_r, rh) awaiting norm_post
            with tc.tile_pool(name="pap", bufs=1, space="PSUM") as pa, \
                 tc.tile_pool(name="osb", bufs=2) as ob:
              for rep in range(reps):
                if rep > 0:
                    xkm = load_x(xk)
                    xqm = load_x(xq)
                    xvm = load_x(xv)
                qT_s = sb.tile([P, 2 * T], BF16)
                kT_s = sb.tile([P, 2 * T], BF16)
                v_s = sb.tile([P, NJ * 260], BF16)
                yT_s = sb.tile([P, 2 * T], BF16)

                if True:

                    def proj_qk_tile(w_m, b_t, x_m, out_s, dt, t0):
                        """One [P, 512] q/k projection tile: 8-chunk matmul
                        accumulate + DVE bias eviction into out_s dt half."""
                        ps = pa.tile([P, 512], F32, tag="aux", bufs=2,
                                     name="pj")
                        for kc in range(KF):
                            lhsT = w_m[:, kc * CS + dt * P: kc * CS + dt * P + P]
                            nc.tensor.matmul(
                                ps, lhsT,
                                x_m[:, kc * T + t0: kc * T + t0 + 512],
                                start=(kc == 0), stop=(kc == KF - 1))
                        nc.scalar.add(
                            out_s[:, dt * T + t0: dt * T + t0 + 512],
                            ps, b_t[:, dt:dt + 1])

                    def proj_v_tile(jt):
                        """One [P, 260] v-projection chunk (tokens jt*P..)."""
                        pv = pa.tile([P, NH * 65], F32, tag="aux", bufs=2,
                                     name="pv")
                        for kc in range(KF):
                            nc.tensor.matmul(
                                pv, xvm[:, kc * T + jt * P: kc * T + (jt + 1) * P],
                                wv_m[:, kc * NH * 65:(kc + 1) * NH * 65],
                                start=(kc == 0), stop=(kc == KF - 1))
                        nc.vector.tensor_add(v_s[:, jt * 260:(jt + 1) * 260],
                                             pv, bv_s)

                    # ---- pre-attention: full q/k projections (k first; its
                    # dt1 half fills PE while the xq DMA stream lands), with
                    # the previous rep's drain units interleaved between tiles
                    projs = [
                        (lambda dt=dt, th=th: proj_qk_tile(
                            wk_m, bk_s, xkm, kT_s, dt, th * 512))
                        for dt in range(2) for th in range(T // 512)
                    ] + [
                        (lambda dt=dt, th=th: proj_qk_tile(
                            wq_m, bq_s, xqm, qT_s, dt, th * 512))
                        for dt in range(2) for th in range(T // 512)
                    ]
                    cdone = 0
                    for i, pt in enumerate(projs):
                        want = (len(carry) * (i + 1)) // len(projs)
                        while cdone < want:
                            carry[cdone]()
                            cdone += 1
                        pt()
                    for u in carry[cdone:]:
                        u()
                    carry = []

                    # ---- PE filler queue: work metered into attention stalls
                    fillers = [lambda jt=jt: proj_v_tile(jt)
                               for jt in range(NJ)]
                    fillers.reverse()      # pop() from the front of the list

                    def emit_filler(n=1):
                        for _ in range(n):
                            if fillers:
                                fillers.pop()()

                    def st_phase(h, jc, between=None):
                        """S^T + exp + mask for (head h, key chunk jc).
                        Returns es_map entry: list of (c0, cw, es_tile)."""
                        ht, hr = h // 2, (h % 2) * 64
                        ic0 = jc // R
                        d0 = (jc % R) * P
                        entries = []
                        for s in range((T - ic0 * ICе + nstrip - 1) // nstrip):
                            if s > 0 and between is not None:
                                between()
                            c0 = ic0 * ICе + s * nstrip
                            cw = min(nstrip, T - c0)
                            ps = pa.tile([P, nstrip], F32, tag="ps", bufs=2,
                                         name="pst")
                            q0 = d0 if s == 0 else 0
                            while q0 < cw:
                                qw = min(512 - q0 % 512, cw - q0)
                                nc.tensor.matmul(
                                    ps[:, q0:q0 + qw],
                                    kT_s[hr:hr + 64, ht * T + jc * P: ht * T + (jc + 1) * P],
                                    qT_s[hr:hr + 64, ht * T + c0 + q0: ht * T + c0 + q0 + qw],
                                    start=True, stop=True)
                                q0 += qw
                            es = ea.tile([P, nstrip], BF16, tag="es", name="es")
                            e0 = d0 if s == 0 else 0
                            nc.scalar.activation(es[:, e0:cw], ps[:, e0:cw],
                                                 EXP, scale=LSCALE)
                            if s == 0 and d0 < ICе:
                                cp_tog[0] ^= 1
                                me = {"pool": nc.gpsimd, "dve": nc.vector}.get(
                                    mask_eng,
                                    nc.vector if cp_tog[0] else nc.gpsimd)
                                me.tensor_mul(
                                    es[:, d0:ICе], es[:, d0:ICе],
                                    msk_s[:, (jc % R) * ICе + d0:(jc % R + 1) * ICе])
                            entries.append((c0, cw, es))
                        return entries

                    def av_piece(h, r, jc, es_map, py_r):
                        """One key chunk's contribution to region r of head h."""
                        m = jc % R
                        vv = v_s[:, jc * 260 + h * 65: jc * 260 + h * 65 + 65]
                        # locate region r's columns in jc's es tiles
                        ecol = None
                        for (c0, cw, es) in es_map[jc]:
                            if c0 <= r * ICе < c0 + cw:
                                ecol = (es, r * ICе - c0)
                                break
                        es, off = ecol
                        if jc // R == r:
                            # diagonal chunk: cols [m*P, ICе); col block m
                            # ends its accumulation here (split stop flag)
                            nc.tensor.matmul(
                                py_r[:, m * P:(m + 1) * P],
                                vv, es[:, off + m * P: off + (m + 1) * P],
                                start=(jc == 0), stop=True)
                            if (m + 1) * P < ICе:
                                nc.tensor.matmul(
                                    py_r[:, (m + 1) * P:ICе],
                                    vv, es[:, off + (m + 1) * P: off + ICе],
                                    start=(jc == 0), stop=False)
                        else:
                            nc.tensor.matmul(
                                py_r[:, 0:ICе],
                                vv, es[:, off: off + ICе],
                                start=(jc == 0), stop=(jc == R * r + R - 1))

                    def norm_pre(py_r):
                        """Reciprocal of region denominator (DVE), issued
                        right after the region's AV so it runs under other
                        PE work."""
                        rh = nrm.tile([1, ICе], F32R, tag="rh", name="rh")
                        with nc.allow_low_precision(reason="f32r row-scale"):
                            nc.vector.reciprocal(rh, py_r[64:65, :])
                        return rh

                    def norm_post(h, r, py_r, rh):
                        """Broadcast 1/denom and scale region r into yT; then
                        kick the output projection half whose heads are done."""
                        ht, hr = h // 2, (h % 2) * 64
                        rb = pa.tile([HD, ICе], F32, tag="aux", bufs=2,
                                     name="rb")
                        nc.tensor.matmul(rb, one_s, rh, start=True, stop=True)
                        rbs = nrm.tile([HD, ICе], F32, tag="rbs", name="rbs")
                        if rbs_eng == "act":
                            nc.scalar.copy(rbs, rb)
                        else:
                            nc.vector.tensor_copy(rbs, rb)
                        b0 = r * ICе
                        nc.vector.tensor_mul(
                            yT_s[hr:hr + 64, ht * T + b0: ht * T + b0 + ICе],
                            py_r[0:64, :], rbs)
                        if h == 3:
                            out_proj_cc(r)

                    cp_tog = [0]

                    def out_proj_cc(cc):
                        """Output projection for token chunk cc (both head
                        pairs accumulated); runs in the carry/drain where the
                        strip engines are idle. Evictions alternate DVE/ACT."""
                        for it in range(cc * ICе // P, (cc + 1) * ICе // P):
                            ot = ob.tile([P, C], BF16, tag="ot", name="ot")
                            for nn in range(2):
                                pot = pa.tile([P, 512], F32, tag="aux",
                                              bufs=2, name="pot")
                                for ct in range(2):
                                    nc.tensor.matmul(
                                        pot,
                                        yT_s[:, ct * T + it * P: ct * T + (it + 1) * P],
                                        wp_s[:, ct * C + nn * 512: ct * C + (nn + 1) * 512],
                                        start=(ct == 0), stop=(ct == 1))
                                cp_tog[0] ^= 1
                                if cp_tog[0]:
                                    nc.scalar.copy(
                                        ot[:, nn * 512:(nn + 1) * 512], pot)
                                else:
                                    nc.vector.tensor_copy(
                                        ot[:, nn * 512:(nn + 1) * 512], pot)
                            nc.sync.dma_start(out[it * P:(it + 1) * P, :], ot)

                    # ---- skewed pipeline over heads ----

                    def make_av_units(ph, pmap):
                        """Work-unit closures for head ph's AV + norms, in
                        accumulation order (region-major, jc ascending)."""
                        units = []
                        state = {}

                        def start_region(r):
                            state[r] = pa.tile([65, ICе], F32, tag="py",
                                               bufs=2, name="py")

                        for r in range(NR):
                            for jc in range(R * (r + 1)):
                                def u(r=r, jc=jc):
                                    if jc == 0:
                                        start_region(r)
                                    av_piece(ph, r, jc, pmap, state[r])
                                units.append(u)

                            def un(r=r):
                                rh = norm_pre(state[r])
                                if pend[0] is not None:
                                    norm_post(*pend[0])
                                pend[0] = (ph, r, state[r], rh)
                            units.append(un)
                        return units

                    prev_units = []
                    for h in range(NH):
                        es_map = {}
                        units = prev_units + fillers[::-1]
                        fillers.clear()
                        done = 0
                        prog = [0]

                        def pump():
                            prog[0] += 1
                            want = (len(units) * prog[0] + NSTEP - 1) // NSTEP
                            while done_[0] < min(want, len(units)):
                                units[done_[0]]()
                                done_[0] += 1

                        done_ = [0]
                        NSTEP = sum((T - (jc // R) * ICе + nstrip - 1) // nstrip
                                    for jc in range(NJ))
                        for jc in range(NJ):
                            es_map[jc] = st_phase(h, jc, between=pump)
                            pump()
                        done = done_[0]
                        for u in units[done:]:
                            u()
                        prev_units = make_av_units(h, es_map)

                    # last head's AV+norm becomes the next rep's preamble
                    # filler (or the final drain on the last rep)
                    def flush_pend():
                        if pend[0] is not None:
                            norm_post(*pend[0])
                            pend[0] = None
                    carry = prev_units + [flush_pend]

              for u in carry:
                  u()

    return nc


def make_core_inputs(query, key, value, Wq, bq, Wk, bk, Wv, bv, Wp, T=T_FULL):
    """Host-side shard prep. Returns list of 8 in_maps (bf16 numpy)."""
    bf = ml_dtypes.bfloat16
    query = np.asarray(query, np.float32)
    key = np.asarray(key, np.float32)
    value = np.asarray(value, np.float32)
    Wq, bq = np.asarray(Wq, np.float32), np.asarray(bq, np.float32)
    Wk, bk = np.asarray(Wk, np.float32), np.asarray(bk, np.float32)
    Wv, bv = np.asarray(Wv, np.float32), np.asarray(bv, np.float32)
    Wp = np.asarray(Wp, np.float32)

    ICе = min(IC, T)
    R = ICе // P
    jj = np.arange(P)[:, None]
    cc = np.arange(ICе)[None, :]
    msk_np = np.concatenate(
        [(cc >= (128 * m + jj)) for m in range(R)], axis=1).astype(bf)
    ones64 = np.ones((1, HD), np.float32)

    xT = {}
    for nm, x in (("q", query), ("k", key), ("v", value)):
        for b in range(B):
            xT[nm, b] = np.ascontiguousarray(x[b].T).astype(bf)

    in_maps = []
    for core in range(N_CORES):
        b, g = core // 4, core % 4
        hs = slice(g * CS, (g + 1) * CS)
        wv_p = np.zeros((C, NH * 65), np.float32)
        bv_p = np.zeros((P, NH * 65), np.float32)
        wv_h = Wv[:, hs]
        for h in range(NH):
            wv_p[:, h * 65:h * 65 + 64] = wv_h[:, h * 64:(h + 1) * 64]
            bv_p[:, h * 65:h * 65 + 64] = bv[hs][h * 64:(h + 1) * 64][None, :]
            bv_p[:, h * 65 + 64] = 1.0
        in_maps.append(dict(
            xq=xT["q", b], xk=xT["k", b], xv=xT["v", b],
            wq=Wq[:, hs].astype(bf), wk=Wk[:, hs].astype(bf),
            wv=wv_p.astype(bf), wp=Wp[hs, :].astype(bf),
            bq2=np.ascontiguousarray(bq[hs].reshape(2, P).T),
            bk2=np.ascontiguousarray(bk[hs].reshape(2, P).T),
            bv260=bv_p, msk=msk_np, ones64=ones64))
    return in_maps


_NC = None
TRACE = False          # set True (e.g. from test.py) to neuron-profile the run
LAST = None            # BassKernelResults of the most recent kernel() call


def kernel(query, key, value, att_mask, Wq, bq, Wk, bk, Wv, bv, Wp, bp):
    from concourse.bass_utils import run_bass_kernel_spmd
    global _NC, LAST
    if _NC is None:
        _NC = build_nc()
        _NC.finalize()
    in_maps = make_core_inputs(query, key, value, Wq, bq, Wk, bk, Wv, bv, Wp)
    try:
        res = run_bass_kernel_spmd(_NC, in_maps, core_ids=list(range(N_CORES)),
                                   trace=TRACE)
    except Exception:
        # transient axon-tunnel desyncs happen; one retry is usually enough
        res = run_bass_kernel_spmd(_NC, in_maps, core_ids=list(range(N_CORES)),
                                   trace=TRACE)
    LAST = res
    full = np.zeros((B, T_FULL, C), np.float32)
    for core in range(N_CORES):
        full[core // 4] += res.results[core]["out"].astype(np.float32)
    full += np.asarray(bp, np.float32)[None, None, :]
    return full


# revision 46
# speedup vs baseline: 1.2074x; 1.0154x over previous
"""Causal multi-head attention (PBrelax) for TRN2, sharded over 8 NeuronCores.

Sharding: batch (2) x head-group (4 heads each) = 8 shards, one per core.
Each core computes q/k/v projections for its 256 channels, causal
flash-style attention in S^T layout (keys on partitions), and a partial
output projection; the host sums the 4 per-batch partials and adds bp.

The global abs-max subtraction in PBrelax is softmax-shift-invariant, so it
is mathematically a no-op on the output; logits are bounded (~|x|<4) so
exp() without max-subtraction is numerically safe.

Schedule (v2): the attention phase is ACT(exp)-bound per strip while the
projections are PE-bound, so the kernel runs a skewed software pipeline:
stream(h) emits head h's S^T+exp+mask interleaved with ready PE work —
head h-1's AV pieces + per-region normalize (region-major, so PSUM holds
one [65,512] region at a time instead of a [65,T] head) and the v
projection (head 0's stream). The output projection is split by head
pair into two bf16 partials summed on host, so its first half runs amid
streams h2/h3. Across reps, the last head's AV/norm/output drain is
carried into the next rep's q/k-projection preamble (cross-rep software
pipelining); x loads are dispatched from the Pool queue so they land
during the previous rep's tail. Causal trimming skips all
below-diagonal strip columns (split stop flags finalize each PSUM
column block at its true last accumulation). Engines: exp on ACT, q/k
bias eviction on ACT, everything PSUM-touching else on DVE (GPSIMD
cannot read PSUM), SBUF-only mask multiplies on Pool.
PSUM budget: ps strips 2x2 banks, py regions 2x1, aux (proj/pv/rb/pot)
2x1 = 8 banks.

fp8 was evaluated and rejected: for random-sign dot products the output
relative error equals the per-element quantization error (~5% for
e4m3, no sqrt-N averaging), which would blow the 2e-2 gate; bf16
(~0.3%) is the floor dtype here.
"""

import numpy as np
import ml_dtypes

import concourse.bass as bass
import concourse.bacc as bacc
import concourse.mybir as mybir
import concourse.tile as tile

BF16 = mybir.dt.bfloat16
F32 = mybir.dt.float32
F32R = mybir.dt.float32r
EXP = mybir.ActivationFunctionType.Exp

B, T_FULL, C, H = 2, 2048, 1024, 16
HD = 64
NH = 4            # heads per core
CS = NH * HD      # 256 channels per core
P = 128
IC = 512          # attention region width (= one PSUM bank of f32)
KF = C // P       # 8 contraction chunks
LSCALE = 0.125    # (1/(alpha*sqrt(hd))) * alpha = 1/8
N_CORES = 8


def build_nc(T=T_FULL, nstrip=1024, reps=1, mask_eng="pool", rbs_eng="dve"):
    NJ = T // P
    ICе = min(IC, T)
    R = ICе // P
    NR = T // ICе     # regions per head
    nc = bacc.Bacc(target_bir_lowering=False)

    xq = nc.dram_tensor("xq", [C, T], BF16, kind="ExternalInput")
    xk = nc.dram_tensor("xk", [C, T], BF16, kind="ExternalInput")
    xv = nc.dram_tensor("xv", [C, T], BF16, kind="ExternalInput")
    wq = nc.dram_tensor("wq", [C, CS], BF16, kind="ExternalInput")
    wk = nc.dram_tensor("wk", [C, CS], BF16, kind="ExternalInput")
    wv = nc.dram_tensor("wv", [C, NH * 65], BF16, kind="ExternalInput")
    wp = nc.dram_tensor("wp", [CS, C], BF16, kind="ExternalInput")
    bq2 = nc.dram_tensor("bq2", [P, 2], F32, kind="ExternalInput")
    bk2 = nc.dram_tensor("bk2", [P, 2], F32, kind="ExternalInput")
    bv260 = nc.dram_tensor("bv260", [P, NH * 65], F32, kind="ExternalInput")
    msk = nc.dram_tensor("msk", [P, R * ICе], BF16, kind="ExternalInput")
    ones64 = nc.dram_tensor("ones64", [1, HD], F32R, kind="ExternalInput")
    out = nc.dram_tensor("out", [T, C], BF16, kind="ExternalOutput")

    with tile.TileContext(nc) as tc:
        with tc.tile_pool(name="sb", bufs=1) as sb, \
             tc.tile_pool(name="xp", bufs=2) as xp, \
             tc.tile_pool(name="es", bufs=2 * NJ) as ea, \
             tc.tile_pool(name="nrm", bufs=3) as nrm:

            def load_x(xd, after_first=None):
                """Load x chunks; optionally emit (weight, bias) DMAs right
                after chunk 0 so the first matmul isn't gated on the full x."""
                xm = xp.tile([P, KF * T], BF16, tag="x", name="xm")
                for kc in range(KF):
                    nc.gpsimd.dma_start(xm[:, kc * T:(kc + 1) * T],
                                        xd[kc * P:(kc + 1) * P, :])
                    if kc == 0 and after_first is not None:
                        after_first()
                return xm

            # ---- weights / constants; xk0+wk+bk first so k-proj starts ASAP
            wk_m = sb.tile([P, KF * CS], BF16)
            bk_d = sb.tile([P, 2], F32)

            def _wk_dmas():
                nc.sync.dma_start(wk_m.rearrange("p (c n) -> p c n", c=KF),
                                  wk[:, :].rearrange("(c p) n -> p c n", p=P))
                nc.sync.dma_start(bk_d, bk2[:, :])
            xkm = load_x(xk, _wk_dmas)
            wq_m = sb.tile([P, KF * CS], BF16)
            bq_d = sb.tile([P, 2], F32)

            def _wq_dmas():
                nc.sync.dma_start(wq_m.rearrange("p (c n) -> p c n", c=KF),
                                  wq[:, :].rearrange("(c p) n -> p c n", p=P))
                nc.sync.dma_start(bq_d, bq2[:, :])
            xqm = load_x(xq, _wq_dmas)
            wv_m = sb.tile([P, KF * NH * 65], BF16)
            bv_d = sb.tile([P, NH * 65], F32)

            def _wv_dmas():
                nc.sync.dma_start(wv_m.rearrange("p (c n) -> p c n", c=KF),
                                  wv[:, :].rearrange("(c p) n -> p c n", p=P))
                nc.sync.dma_start(bv_d, bv260[:, :])
            xvm = load_x(xv, _wv_dmas)
            msk_d = sb.tile([P, R * ICе], BF16)
            nc.sync.dma_start(msk_d, msk[:, :])
            wp_s = sb.tile([P, 2 * C], BF16)
            nc.sync.dma_start(wp_s.rearrange("p (c n) -> p c n", c=2),
                              wp[:, :].rearrange("(c p) n -> p c n", p=P))
            one_s = sb.tile([1, HD], F32R)
            nc.sync.dma_start(one_s, ones64[:, :])
            # prime the ACT exp table during the DMA preamble so the first
            # attention strip doesn't pay the 1.3us table load
            warm = sb.tile([1, 2], F32)
            nc.vector.memset(warm, 0.0)
            warm2 = sb.tile([1, 2], BF16)
            nc.scalar.activation(warm2, warm, EXP, scale=1.0)
            # pre-touch constants on DVE so downstream DVE consumers need no
            # extra cross-engine waits (walrus sync-wait slots are scarce)
            bq_s = sb.tile([P, 2], F32)
            nc.vector.tensor_copy(bq_s, bq_d)
            bk_s = sb.tile([P, 2], F32)
            nc.vector.tensor_copy(bk_s, bk_d)
            bv_s = sb.tile([P, NH * 65], F32)
            nc.vector.tensor_copy(bv_s, bv_d)
            msk_s = sb.tile([P, R * ICе], BF16)
            nc.vector.tensor_copy(msk_s, msk_d)

            carry = []       # previous rep's drain units (run in our preamble)
            pend = [None]    # (h, r, py_r, rh) awaiting norm_post
            with tc.tile_pool(name="pap", bufs=1, space="PSUM") as pa, \
                 tc.tile_pool(name="osb", bufs=4) as ob:
              nxt = {}
              for rep in range(reps):
                if rep > 0:
                    xkm = nxt.pop("k") if "k" in nxt else load_x(xk)
                    xqm = nxt.pop("q") if "q" in nxt else load_x(xq)
                    xvm = load_x(xv)
                qT_s = sb.tile([P, 2 * T], BF16)
                kT_s = sb.tile([P, 2 * T], BF16)
                v_s = sb.tile([P, NJ * 260], BF16)
                yT_s = sb.tile([P, 2 * T], BF16)

                if True:

                    def proj_qk_tile(w_m, b_t, x_m, out_s, dt, t0):
                        """One [P, 512] q/k projection tile: 8-chunk matmul
                        accumulate + DVE bias eviction into out_s dt half."""
                        ps = pa.tile([P, 512], F32, tag="aux", bufs=2,
                                     name="pj")
                        for kc in range(KF):
                            lhsT = w_m[:, kc * CS + dt * P: kc * CS + dt * P + P]
                            nc.tensor.matmul(
                                ps, lhsT,
                                x_m[:, kc * T + t0: kc * T + t0 + 512],
                                start=(kc == 0), stop=(kc == KF - 1))
                        nc.scalar.add(
                            out_s[:, dt * T + t0: dt * T + t0 + 512],
                            ps, b_t[:, dt:dt + 1])

                    def proj_v_tile(jt):
                        """One [P, 260] v-projection chunk (tokens jt*P..)."""
                        pv = pa.tile([P, NH * 65], F32, tag="aux", bufs=2,
                                     name="pv")
                        for kc in range(KF):
                            nc.tensor.matmul(
                                pv, xvm[:, kc * T + jt * P: kc * T + (jt + 1) * P],
                                wv_m[:, kc * NH * 65:(kc + 1) * NH * 65],
                                start=(kc == 0), stop=(kc == KF - 1))
                        nc.vector.tensor_add(v_s[:, jt * 260:(jt + 1) * 260],
                                             pv, bv_s)

                    # ---- pre-attention: full q/k projections (k first; its
                    # dt1 half fills PE while the xq DMA stream lands), with
                    # the previous rep's drain units interleaved between tiles
                    projs = [
                        (lambda dt=dt, th=th: proj_qk_tile(
                            wk_m, bk_s, xkm, kT_s, dt, th * 512))
                        for dt in range(2) for th in range(T // 512)
                    ] + [
                        (lambda dt=dt, th=th: proj_qk_tile(
                            wq_m, bq_s, xqm, qT_s, dt, th * 512))
                        for dt in range(2) for th in range(T // 512)
                    ]
                    cdone = 0
                    for i, pt in enumerate(projs):
                        want = (len(carry) * (i + 1)) // len(projs)
                        while cdone < want:
                            carry[cdone]()
                            cdone += 1
                        pt()
                    for u in carry[cdone:]:
                        u()
                    carry = []

                    # ---- PE filler queue: work metered into attention stalls
                    fillers = [lambda jt=jt: proj_v_tile(jt)
                               for jt in range(NJ)]
                    fillers.reverse()      # pop() from the front of the list

                    def emit_filler(n=1):
                        for _ in range(n):
                            if fillers:
                                fillers.pop()()

                    def st_phase(h, jc, between=None):
                        """S^T + exp + mask for (head h, key chunk jc).
                        Returns es_map entry: list of (c0, cw, es_tile)."""
                        ht, hr = h // 2, (h % 2) * 64
                        ic0 = jc // R
                        d0 = (jc % R) * P
                        entries = []
                        for s in range((T - ic0 * ICе + nstrip - 1) // nstrip):
                            if s > 0 and between is not None:
                                between()
                            c0 = ic0 * ICе + s * nstrip
                            cw = min(nstrip, T - c0)
                            ps = pa.tile([P, nstrip], F32, tag="ps", bufs=2,
                                         name="pst")
                            q0 = d0 if s == 0 else 0
                            while q0 < cw:
                                qw = min(512 - q0 % 512, cw - q0)
                                nc.tensor.matmul(
                                    ps[:, q0:q0 + qw],
                                    kT_s[hr:hr + 64, ht * T + jc * P: ht * T + (jc + 1) * P],
                                    qT_s[hr:hr + 64, ht * T + c0 + q0: ht * T + c0 + q0 + qw],
                                    start=True, stop=True)
                                q0 += qw
                            es = ea.tile([P, nstrip], BF16, tag="es", name="es")
                            e0 = d0 if s == 0 else 0
                            nc.scalar.activation(es[:, e0:cw], ps[:, e0:cw],
                                                 EXP, scale=LSCALE)
                            if s == 0 and d0 < ICе:
                                cp_tog[0] ^= 1
                                me = {"pool": nc.gpsimd, "dve": nc.vector}.get(
                                    mask_eng,
                                    nc.vector if cp_tog[0] else nc.gpsimd)
                                me.tensor_mul(
                                    es[:, d0:ICе], es[:, d0:ICе],
                                    msk_s[:, (jc % R) * ICе + d0:(jc % R + 1) * ICе])
                            entries.append((c0, cw, es))
                        return entries

                    def av_piece(h, r, jc, es_map, py_r):
                        """One key chunk's contribution to region r of head h."""
                        m = jc % R
                        vv = v_s[:, jc * 260 + h * 65: jc * 260 + h * 65 + 65]
                        # locate region r's columns in jc's es tiles
                        ecol = None
                        for (c0, cw, es) in es_map[jc]:
                            if c0 <= r * ICе < c0 + cw:
                                ecol = (es, r * ICе - c0)
                                break
                        es, off = ecol
                        if jc // R == r:
                            # diagonal chunk: cols [m*P, ICе); col block m
                            # ends its accumulation here (split stop flag)
                            nc.tensor.matmul(
                                py_r[:, m * P:(m + 1) * P],
                                vv, es[:, off + m * P: off + (m + 1) * P],
                                start=(jc == 0), stop=True)
                            if (m + 1) * P < ICе:
                                nc.tensor.matmul(
                                    py_r[:, (m + 1) * P:ICе],
                                    vv, es[:, off + (m + 1) * P: off + ICе],
                                    start=(jc == 0), stop=False)
                        else:
                            nc.tensor.matmul(
                                py_r[:, 0:ICе],
                                vv, es[:, off: off + ICе],
                                start=(jc == 0), stop=(jc == R * r + R - 1))

                    def norm_pre(py_r):
                        """Reciprocal of region denominator (DVE), issued
                        right after the region's AV so it runs under other
                        PE work."""
                        rh = nrm.tile([1, ICе], F32R, tag="rh", name="rh")
                        with nc.allow_low_precision(reason="f32r row-scale"):
                            nc.vector.reciprocal(rh, py_r[64:65, :])
                        return rh

                    def norm_post(h, r, py_r, rh):
                        """Broadcast 1/denom and scale region r into yT; then
                        kick the output projection half whose heads are done."""
                        ht, hr = h // 2, (h % 2) * 64
                        rb = pa.tile([HD, ICе], F32, tag="aux", bufs=2,
                                     name="rb")
                        nc.tensor.matmul(rb, one_s, rh, start=True, stop=True)
                        rbs = nrm.tile([HD, ICе], F32, tag="rbs", name="rbs")
                        if rbs_eng == "act":
                            nc.scalar.copy(rbs, rb)
                        else:
                            nc.vector.tensor_copy(rbs, rb)
                        b0 = r * ICе
                        nc.vector.tensor_mul(
                            yT_s[hr:hr + 64, ht * T + b0: ht * T + b0 + ICе],
                            py_r[0:64, :], rbs)
                        if h == 3:
                            out_proj_cc(r)

                    cp_tog = [0]

                    def out_proj_cc(cc):
                        """Output projection for token chunk cc (both head
                        pairs accumulated); runs in the carry/drain where the
                        strip engines are idle. Evictions alternate DVE/ACT."""
                        for it in range(cc * ICе // P, (cc + 1) * ICе // P):
                            ot = ob.tile([P, C], BF16, tag="ot", name="ot")
                            for nn in range(2):
                                pot = pa.tile([P, 512], F32, tag="ps",
                                              bufs=2, name="pot")
                                for ct in range(2):
                                    nc.tensor.matmul(
                                        pot,
                                        yT_s[:, ct * T + it * P: ct * T + (it + 1) * P],
                                        wp_s[:, ct * C + nn * 512: ct * C + (nn + 1) * 512],
                                        start=(ct == 0), stop=(ct == 1))
                                cp_tog[0] ^= 1
                                if cp_tog[0]:
                                    nc.scalar.copy(
                                        ot[:, nn * 512:(nn + 1) * 512], pot)
                                else:
                                    nc.vector.tensor_copy(
                                        ot[:, nn * 512:(nn + 1) * 512], pot)
                            nc.sync.dma_start(out[it * P:(it + 1) * P, :], ot)

                    # ---- skewed pipeline over heads ----

                    def make_av_units(ph, pmap):
                        """Work-unit closures for head ph's AV + norms, in
                        accumulation order (region-major, jc ascending)."""
                        units = []
                        state = {}

                        def start_region(r):
                            state[r] = pa.tile([65, ICе], F32, tag="py",
                                               bufs=2, name="py")

                        for r in range(NR):
                            for jc in range(R * (r + 1)):
                                def u(r=r, jc=jc):
                                    if jc == 0:
                                        start_region(r)
                                    av_piece(ph, r, jc, pmap, state[r])
                                units.append(u)

                            def un(r=r):
                                rh = norm_pre(state[r])
                                if pend[0] is not None:
                                    norm_post(*pend[0])
                                pend[0] = (ph, r, state[r], rh)
                            units.append(un)
                        return units

                    prev_units = []
                    for h in range(NH):
                        if h == NH - 1 and rep + 1 < reps:
                            # prefetch next rep's k/q inputs: their buffers
                            # free early (q/v-proj of this rep are done), so
                            # the loads land during this rep's tail. xv is NOT
                            # safe (its buffer waits next rep's k-proj).
                            nxt["k"] = load_x(xk)
                            nxt["q"] = load_x(xq)
                        es_map = {}
                        units = prev_units + fillers[::-1]
                        fillers.clear()
                        done = 0
                        prog = [0]

                        def pump():
                            prog[0] += 1
                            want = (len(units) * prog[0] + NSTEP - 1) // NSTEP
                            while done_[0] < min(want, len(units)):
                                units[done_[0]]()
                                done_[0] += 1

                        done_ = [0]
                        NSTEP = sum((T - (jc // R) * ICе + nstrip - 1) // nstrip
                                    for jc in range(NJ))
                        for jc in range(NJ):
                            es_map[jc] = st_phase(h, jc, between=pump)
                            pump()
                        done = done_[0]
                        for u in units[done:]:
                            u()
                        prev_units = make_av_units(h, es_map)

                    # last head's AV+norm becomes the next rep's preamble
                    # filler (or the final drain on the last rep)
                    def flush_pend():
                        if pend[0] is not None:
                            norm_post(*pend[0])
                            pend[0] = None
                    carry = prev_units + [flush_pend]

              for u in carry:
                  u()

    return nc


def make_core_inputs(query, key, value, Wq, bq, Wk, bk, Wv, bv, Wp, T=T_FULL):
    """Host-side shard prep. Returns list of 8 in_maps (bf16 numpy)."""
    bf = ml_dtypes.bfloat16
    query = np.asarray(query, np.float32)
    key = np.asarray(key, np.float32)
    value = np.asarray(value, np.float32)
    Wq, bq = np.asarray(Wq, np.float32), np.asarray(bq, np.float32)
    Wk, bk = np.asarray(Wk, np.float32), np.asarray(bk, np.float32)
    Wv, bv = np.asarray(Wv, np.float32), np.asarray(bv, np.float32)
    Wp = np.asarray(Wp, np.float32)

    ICе = min(IC, T)
    R = ICе // P
    jj = np.arange(P)[:, None]
    cc = np.arange(ICе)[None, :]
    msk_np = np.concatenate(
        [(cc >= (128 * m + jj)) for m in range(R)], axis=1).astype(bf)
    ones64 = np.ones((1, HD), np.float32)

    xT = {}
    for nm, x in (("q", query), ("k", key), ("v", value)):
        for b in range(B):
            xT[nm, b] = np.ascontiguousarray(x[b].T).astype(bf)

    in_maps = []
    for core in range(N_CORES):
        b, g = core // 4, core % 4
        hs = slice(g * CS, (g + 1) * CS)
        wv_p = np.zeros((C, NH * 65), np.float32)
        bv_p = np.zeros((P, NH * 65), np.float32)
        wv_h = Wv[:, hs]
        for h in range(NH):
            wv_p[:, h * 65:h * 65 + 64] = wv_h[:, h * 64:(h + 1) * 64]
            bv_p[:, h * 65:h * 65 + 64] = bv[hs][h * 64:(h + 1) * 64][None, :]
            bv_p[:, h * 65 + 64] = 1.0
        in_maps.append(dict(
            xq=xT["q", b], xk=xT["k", b], xv=xT["v", b],
            wq=Wq[:, hs].astype(bf), wk=Wk[:, hs].astype(bf),
            wv=wv_p.astype(bf), wp=Wp[hs, :].astype(bf),
            bq2=np.ascontiguousarray(bq[hs].reshape(2, P).T),
            bk2=np.ascontiguousarray(bk[hs].reshape(2, P).T),
            bv260=bv_p, msk=msk_np, ones64=ones64))
    return in_maps


_NC = None
TRACE = False          # set True (e.g. from test.py) to neuron-profile the run
LAST = None            # BassKernelResults of the most recent kernel() call


def kernel(query, key, value, att_mask, Wq, bq, Wk, bk, Wv, bv, Wp, bp):
    from concourse.bass_utils import run_bass_kernel_spmd
    global _NC, LAST
    if _NC is None:
        _NC = build_nc()
        _NC.finalize()
    in_maps = make_core_inputs(query, key, value, Wq, bq, Wk, bk, Wv, bv, Wp)
    try:
        res = run_bass_kernel_spmd(_NC, in_maps, core_ids=list(range(N_CORES)),
                                   trace=TRACE)
    except Exception:
        # transient axon-tunnel desyncs happen; one retry is usually enough
        res = run_bass_kernel_spmd(_NC, in_maps, core_ids=list(range(N_CORES)),
                                   trace=TRACE)
    LAST = res
    full = np.zeros((B, T_FULL, C), np.float32)
    for core in range(N_CORES):
        full[core // 4] += res.results[core]["out"].astype(np.float32)
    full += np.asarray(bp, np.float32)[None, None, :]
    return full
